# revision 9
# baseline (speedup 1.0000x reference)
"""Trainium2 Bass kernel for nn_AbstractConv3D (16-level 3x3x3 conv, 16ch).

Strategy (per core, uniform SPMD over 8 cores):
  - Host pads each level's voxel grid (x,y by 1 + x rounded up to the
    window grid, z sharded into 8 slabs with 1-plane halo) so the device
    kernel sees dense zero-padded grids and needs no boundary logic.
  - Device: per (level, batch): DMA rows in natural layout, PE-transpose
    overlapping 8-voxel windows into a (window*ci, site) "T" layout, then
    banded matmuls: lhsT = banded weights [K=128=(8vox x 16ci),
    M=96=(6 out x 16co)], rhs = T columns, 9 (dz,dy) taps accumulated in
    PSUM (float32r = full-rate PE).  Output [96, sites] is PE-transposed
    back to site-major and DMA'd out.  Bias is fused into the PSUM->SBUF
    copy on the scalar engine.
"""

import math

import numpy as np

import concourse.bass as bass
import concourse.tile as tile
from concourse import bacc, mybir
from concourse.masks import make_identity

NUM_LEVELS = 16
C = 16
B = 2
N_CORES = 8
F32 = mybir.dt.float32
F32R = mybir.dt.float32r

# Banded-matmul geometry: window = 8 voxels (K = 8*16 = 128), 6 outputs
# per window (M = 6*16 = 96), windows at stride 6 voxels.
WIN = 8
G = 6


class _LevelGeom:
    def __init__(self, R):
        self.R = R
        self.S = math.ceil(R / N_CORES)          # output z-planes per core
        self.nblk = math.ceil(R / G)             # windows per row
        self.XP = G * self.nblk + 2              # padded x extent (voxels)
        self.YP = R + 2                          # padded y extent (rows/plane)
        self.ZP = self.S + 2                     # input z-planes per core slab
        self.RW = self.XP * C                    # floats per padded row
        self.rows = self.ZP * self.YP            # input rows per (core, batch)
        self.colsT = self.rows * self.nblk       # T columns per (core, batch)
        self.q = 128 // self.nblk                # whole rows per out-transpose
        self.CH = self.q * self.nblk             # sites per out-transpose chunk
        self.NMM = (512 // self.CH) * self.CH    # sites per matmul group
        self.in_floats = B * self.rows * self.RW
        self.out_rows = self.S * self.YP         # output rows per (core, batch)
        self.out_floats = B * self.out_rows * self.RW


def _configure(resolutions):
    global RESOLUTIONS, GEOMS, _IN_OFF, _OUT_OFF, TOTAL_IN, TOTAL_OUT
    global _LVL_OFF, NUM_LEVELS, _CACHED_NC
    RESOLUTIONS = list(resolutions)
    NUM_LEVELS = len(RESOLUTIONS)
    GEOMS = [_LevelGeom(R) for R in RESOLUTIONS]
    _IN_OFF = np.concatenate(
        [[0], np.cumsum([g.in_floats for g in GEOMS])]).astype(int)
    _OUT_OFF = np.concatenate(
        [[0], np.cumsum([g.out_floats for g in GEOMS])]).astype(int)
    TOTAL_IN = int(_IN_OFF[-1])
    TOTAL_OUT = int(_OUT_OFF[-1])
    _LVL_OFF = np.concatenate(
        [[0], np.cumsum([r ** 3 for r in RESOLUTIONS])]).astype(int)
    _CACHED_NC = None


_CACHED_NC = None
_configure([16, 18, 20, 22, 24, 27, 30, 34, 38, 42, 47, 52, 58, 64, 72, 80])

# --------------------------------------------------------------------------
# Device program
# --------------------------------------------------------------------------

def build_nc():
    nc = bacc.Bacc("TRN2", target_bir_lowering=False, debug=False,
                   num_devices=N_CORES)
    xin_h = nc.dram_tensor("xin", [TOTAL_IN], F32, kind="ExternalInput")
    xout_h = nc.dram_tensor("xout", [TOTAL_OUT], F32, kind="ExternalOutput")
    wband_h = nc.dram_tensor("wband", [NUM_LEVELS, 128, 9 * 96], F32,
                             kind="ExternalInput")
    biasv_h = nc.dram_tensor("biasv", [NUM_LEVELS, 96, 1], F32,
                             kind="ExternalInput")
    xin, xout, wband, biasv = (h.ap() for h in
                               (xin_h, xout_h, wband_h, biasv_h))

    with tile.TileContext(nc) as tc:
        with (
            tc.tile_pool(name="const", bufs=1) as cpool,
            tc.tile_pool(name="wb", bufs=2) as wpool,
            tc.tile_pool(name="a", bufs=3) as apool,
            tc.tile_pool(name="t", bufs=2) as tpool,
            tc.tile_pool(name="o1", bufs=8) as o1pool,
            tc.tile_pool(name="o2", bufs=3) as o2pool,
            tc.tile_pool(name="psin", bufs=2, space="PSUM") as psin_pool,
            tc.tile_pool(name="psmm", bufs=2, space="PSUM") as psmm_pool,
            tc.tile_pool(name="psout", bufs=2, space="PSUM") as psout_pool,
        ):
            ident = cpool.tile([128, 128], F32)
            make_identity(nc, ident)

            for l in range(NUM_LEVELS):
                g = GEOMS[l]
                nblk, YP, RW, rows = g.nblk, g.YP, g.RW, g.rows
                S = g.S

                wb_ld = wpool.tile([128, 9 * 96], F32, tag="wb_ld")
                nc.sync.dma_start(wb_ld[:], wband[l])
                wb = wpool.tile([128, 9 * 96], F32R, tag="wb")
                nc.vector.tensor_copy(wb[:], wb_ld[:])
                bv = wpool.tile([96, 1], F32, tag="bv")
                nc.sync.dma_start(bv[:], biasv[l])

                for b in range(B):
                    # ---- phase A: load + transpose into T (window-major) ----
                    Ta = tpool.tile([128, nblk * rows + 2], F32R, tag="T")
                    nc.gpsimd.memset(Ta[:, 0:1].bitcast(F32), 0.0)
                    nc.gpsimd.memset(Ta[:, -1:].bitcast(F32), 0.0)
                    T3 = Ta[:, 1:1 + nblk * rows].rearrange(
                        "p (n r) -> p n r", r=rows)

                    base = int(_IN_OFF[l]) + b * rows * RW
                    src = xin[base:base + rows * RW].rearrange(
                        "(r f) -> r f", f=RW)
                    for c0 in range(0, rows, 128):
                        rc = min(128, rows - c0)
                        A = apool.tile([128, RW], F32, tag="A")
                        nc.sync.dma_start(A[:rc], src[c0:c0 + rc])
                        for n in range(nblk):
                            ps = psin_pool.tile([128, 128], F32, tag="psin")
                            nc.tensor.transpose(
                                ps[:, :rc], A[:rc, 96 * n:96 * n + 128],
                                ident[:rc, :rc])
                            nc.vector.tensor_copy(
                                T3[:, n, c0:c0 + rc], ps[:, :rc])

                    # ---- phase B: banded matmuls + output ----
                    # output rows r in [YP, (S+1)*YP); window bundles of <=4
                    obase = int(_OUT_OFF[l]) + b * g.out_rows * RW
                    orows = S * YP
                    for n0 in range(0, nblk, 4):
                        bw = min(4, nblk - n0)
                        for r0 in range(0, orows, 512):
                            N = min(512, orows - r0)
                            O1s = []
                            for w in range(bw):
                                cb = 1 + (n0 + w) * rows + YP + r0
                                P = psmm_pool.tile([96, N], F32, tag="psmm",
                                                   padded_shape=[96, 512])
                                for t in range(9):
                                    sh = (t // 3 - 1) * YP + (t % 3 - 1)
                                    rhs = Ta[:, cb + sh: cb + sh + N]
                                    nc.tensor.matmul(
                                        P[:], wb[:, t * 96:(t + 1) * 96], rhs,
                                        start=(t == 0), stop=(t == 8))
                                O1 = o1pool.tile([96, N], F32, tag="O1",
                                                 padded_shape=[96, 512])
                                nc.scalar.activation(
                                    O1[:], P[:],
                                    mybir.ActivationFunctionType.Identity,
                                    bias=bv[:])
                                O1s.append(O1)
                            # out-transpose + store, 128-row chunks
                            for c0 in range(0, N, 128):
                                cw = min(128, N - c0)
                                ps2 = psout_pool.tile([128, bw * 96], F32,
                                                      tag="psout",
                                                      padded_shape=[128, 4 * 96])
                                for w in range(bw):
                                    nc.tensor.transpose(
                                        ps2[:cw, 96 * w:96 * w + 96],
                                        O1s[w][:, c0:c0 + cw], ident[:96, :96])
                                O2 = o2pool.tile([128, bw * 96], F32, tag="O2",
                                                 padded_shape=[128, 4 * 96])
                                nc.scalar.copy(O2[:cw], ps2[:cw])
                                rr = r0 + c0
                                rv = xout[obase + rr * RW:
                                          obase + (rr + cw) * RW].rearrange(
                                    "(r f) -> r f", f=RW)
                                dst = rv[:, C + 96 * n0: C + 96 * (n0 + bw)]
                                nc.sync.dma_start(dst, O2[:cw])
    nc.compile()
    return nc


# --------------------------------------------------------------------------
# Host side: padding, weight banding, shard/unshard
# --------------------------------------------------------------------------

def _build_wband(weight):
    """weight: (L, 3, 3, 3, Cin, Cout) -> wband (L, 128, 9*96) where
    wband[l, (i*16+ci), (t*96 + g*16+co)] = weight[l, kd, kh, kw, ci, co]
    for t = kd*3+kh, i = g+kw (0 <= i-g <= 2), else 0."""
    L = NUM_LEVELS
    wb = np.zeros((L, 9, WIN, C, G, C), dtype=np.float32)
    w = np.asarray(weight, dtype=np.float32).reshape(L, 9, 3, C, C)
    for gg in range(G):
        for kw in range(3):
            wb[:, :, gg + kw, :, gg, :] += w[:, :, kw, :, :]
    wb = wb.transpose(0, 2, 3, 1, 4, 5).reshape(L, WIN * C, 9 * G * C)
    return np.ascontiguousarray(wb)


def _shard_inputs(input_np):
    """Build per-core padded 1-D input buffers."""
    inp = np.asarray(input_np, dtype=np.float32)
    bufs = [np.zeros(TOTAL_IN, dtype=np.float32) for _ in range(N_CORES)]
    for l, g in enumerate(GEOMS):
        R = g.R
        lvl = inp[:, _LVL_OFF[l]:_LVL_OFF[l + 1]].reshape(B, R, R, R, C)
        for c in range(N_CORES):
            zlo = c * g.S - 1
            slab = np.zeros((B, g.ZP, g.YP, g.XP, C), dtype=np.float32)
            src_lo = max(0, zlo)
            src_hi = min(R, zlo + g.ZP)
            if src_hi > src_lo:
                slab[:, src_lo - zlo:src_hi - zlo, 1:R + 1, 1:R + 1] = \
                    lvl[:, src_lo:src_hi]
            bufs[c][_IN_OFF[l]:_IN_OFF[l + 1]] = slab.reshape(-1)
    return bufs


def _gather_outputs(outs):
    """Per-core 1-D xout buffers -> full (B, N, C) output."""
    total = np.empty((B, int(_LVL_OFF[-1]), C), dtype=np.float32)
    for l, g in enumerate(GEOMS):
        R = g.R
        lvl = np.empty((B, R, R, R, C), dtype=np.float32)
        for c in range(N_CORES):
            nz = min(g.S, R - c * g.S)
            if nz <= 0:
                continue
            arr = np.asarray(outs[c][_OUT_OFF[l]:_OUT_OFF[l + 1]]).reshape(
                B, g.S, g.YP, g.XP, C)
            lvl[:, c * g.S:c * g.S + nz] = arr[:, :nz, 1:R + 1, 1:R + 1]
        total[:, _LVL_OFF[l]:_LVL_OFF[l + 1]] = lvl.reshape(B, R ** 3, C)
    return total


def _get_nc():
    global _CACHED_NC
    if _CACHED_NC is None:
        _CACHED_NC = build_nc()
    return _CACHED_NC


def make_in_maps(input, weight, bias):
    wb = _build_wband(weight)
    bv = np.ascontiguousarray(
        np.tile(np.asarray(bias, np.float32), (1, G))[:, :, None])
    bufs = _shard_inputs(input)
    return [
        {"xin": bufs[c], "wband": wb, "biasv": bv}
        for c in range(N_CORES)
    ]


def kernel(input, weight, bias, offsets, resolutions):
    from concourse import bass2jax
    nc = _get_nc()
    in_maps = make_in_maps(input, weight, bias)
    results = bass2jax.run_bass_via_pjrt(nc, in_maps, n_cores=N_CORES)
    outs = [results[c]["xout"] for c in range(N_CORES)]
    return _gather_outputs(outs)
